# revision 2
# baseline (speedup 1.0000x reference)
"""Trainium2 Bass kernel for the BuseE hyperbolic KG-embedding scorer.

Dense-matmul reformulation (no gathers on device):

  score(b,e) = C_b + sig_b*LE_e + bias_tail_e - ln(max(n2(b,e), MIN))
  n2(b,e)    = |head_b - tail_e|^2 = s_h + t2_e - <head_b, A_e*x_e>

All O(B*D)/O(N_ENT*D) per-row constants (head transform chain, tanh
factors, log terms) are host-precomputed.  The device computes only the
O(B*N*D) term: a pure K=64 dot-product matmul with fp8 operands
  w_b  = fp8(-90.5*head_b)   (stationary, duplicated on both PE halves)
  xq_e = fp8(90.5*A_e*x_e)   (moving)
row-tiled 2x via tile_position (0,0)/(64,0) so two K=64 matmuls run
concurrently in the 128x128 array.  psum = -8192*<h, A_e x_e> is
naturally centered near 0, so a pure-copy epilogue (ACT Copy / DVE
tensor_copy, split across both engines in unit pairs) casts f32 psum ->
fp8 SBUF with minimal quantization loss; the per-b (8192*s_h) and
per-e (8192*t2_e) constants are added back on the host after selection.

Sharding: entities row-sharded 8 ways.  Per batch-tile of 128 rows only
the ~12k unique entities that tile references in the core's range are
scored (host builds per-(core,tile) sorted unique column lists).  Host
selects the needed 1M of ~100M scores via searchsorted, applies ln +
affine terms.
"""

import math

import numpy as np
import ml_dtypes

import concourse.bacc as bacc
import concourse.mybir as mybir
import concourse.tile as tile
from concourse import bass_utils

F32 = mybir.dt.float32
FP8 = mybir.dt.float8e4
AF = mybir.ActivationFunctionType

MIN_NORM = 1e-15
MARGIN = 9.0
N_ENT, N_REL, D = 200000, 500, 64
B, NCAND = 1024, 1024
NCORES = 8
SHARD = N_ENT // NCORES      # 25000 entities per core
P = 128                      # batch rows per tile == psum partitions
NBT = B // P                 # 8 batch tiles, every core runs all of them
SCALE = 8192.0               # s1*s2; psum = -SCALE*<h,Ax>
SQS = math.sqrt(SCALE)       # 90.51, split between the two fp8 operands
GW = 2048                    # l_bt rounding granularity

E4M3 = ml_dtypes.float8_e4m3

_CACHE: dict = {}


def _build(l_bt):
    nc = bacc.Bacc(
        "TRN2",
        target_bir_lowering=False,
        debug=False,
        enable_asserts=False,
        num_devices=NCORES,
    )
    LHS = nc.dram_tensor("lhs", [P, B], FP8, kind="ExternalInput")
    RHS = nc.dram_tensor("rhs", [NBT * P, l_bt // 2], FP8, kind="ExternalInput")
    OUT = nc.dram_tensor("out", [NBT * P, l_bt], FP8, kind="ExternalOutput")

    UW = 1024                      # unit width: one row-tiled MM pair
    with tile.TileContext(nc) as tc:
        with (
            tc.tile_pool(name="w", bufs=1) as wp,
            tc.tile_pool(name="rp", bufs=NBT) as rp,
            tc.tile_pool(name="ppa", bufs=2, space="PSUM") as ppa,
            tc.tile_pool(name="ppv", bufs=2, space="PSUM") as ppv,
            tc.tile_pool(name="opa", bufs=4) as opa,
            tc.tile_pool(name="opv", bufs=4) as opv,
        ):
            # preload every batch-tile's rhs up front: no mid-kernel input
            # DMAs left to get head-of-line blocked behind output DMAs.
            # rhs0 lands in quarters so the first matmuls start ~4us
            # sooner; the rest in halves.
            lhs_sb = wp.tile([P, B], FP8)
            nc.sync.dma_start(lhs_sb[:], LHS[:])
            hw = l_bt // 2
            rhs_tiles = []
            for t in range(NBT):
                rhs_sb = rp.tile([P, hw], FP8, tag="rhs", name=f"rhs{t}")
                nch = 4 if t == 0 else 2
                for k in range(nch):
                    nc.sync.dma_start(
                        rhs_sb[:, k * hw // nch:(k + 1) * hw // nch],
                        RHS[t * P:(t + 1) * P, k * hw // nch:(k + 1) * hw // nch],
                    )
                rhs_tiles.append(rhs_sb)
            gu = 0
            for t in range(NBT):
                rhs_sb = rhs_tiles[t]
                lo = lhs_sb[0:64, t * P:(t + 1) * P]
                hi = lhs_sb[64:128, t * P:(t + 1) * P]
                npairs_bt = l_bt // UW // 2
                ob = None
                for q in range(l_bt // UW):
                    # two independent chains, assigned in contiguous PAIRS
                    # of units (ACT: units 0,1; DVE: 2,3; ...).  Each chain
                    # has its own psum/ob buffers; each epilogue gates on
                    # only its own MM pair.  One merged 2048-col out-DMA
                    # per unit pair, all on the Pool (SWDGE) queue, so the
                    # Sync queue carries nothing but the input preloads.
                    pp = t * npairs_bt + q // 2
                    # alternate pairs; the final pair goes to ACT (the
                    # faster engine) to balance chain totals + the tail
                    act = (pp % 2 == 0) or (pp == NBT * npairs_bt - 1)
                    half = q % 2
                    pool, obp = (ppa, opa) if act else (ppv, opv)
                    ps = pool.tile([P, UW], F32, tag="ps", name=f"ps{t}_{q}")
                    nc.tensor.matmul(
                        ps[:, 0:512],
                        lo, rhs_sb[0:64, q * 512:(q + 1) * 512],
                        start=True, stop=True, tile_position=(0, 0),
                    )
                    nc.tensor.matmul(
                        ps[:, 512:1024],
                        hi, rhs_sb[64:128, q * 512:(q + 1) * 512],
                        start=True, stop=True, tile_position=(64, 0),
                    )
                    if half == 0:
                        ob = obp.tile([P, 2 * UW], FP8, tag="ob", name=f"ob{t}_{q}")
                    dst = ob[:, half * UW:(half + 1) * UW]
                    if act:
                        nc.scalar.activation(dst, ps[:], AF.Copy)
                    else:
                        nc.vector.tensor_copy(dst, ps[:])
                    if half == 1:
                        nc.gpsimd.dma_start(
                            OUT[t * P:(t + 1) * P, (q - 1) * UW:(q + 1) * UW],
                            ob[:],
                        )

    nc.compile()
    return nc


def get_module(l_bt):
    if l_bt not in _CACHE:
        _CACHE[l_bt] = _build(l_bt)
    return _CACHE[l_bt]


def _expmap0(x):
    un = np.maximum(np.linalg.norm(x, axis=-1, keepdims=True), MIN_NORM)
    return np.tanh(un) * x / un


def _mobius(x, y):
    x2 = np.sum(x * x, -1, keepdims=True)
    y2 = np.sum(y * y, -1, keepdims=True)
    xy = np.sum(x * y, -1, keepdims=True)
    num = (1 + 2 * xy + y2) * x + (1 - x2) * y
    den = 1 + 2 * xy + x2 * y2
    return num / np.maximum(den, MIN_NORM)


def _givens(rd, x):
    g = rd.reshape(rd.shape[:-1] + (-1, 2))
    g = g / np.maximum(np.linalg.norm(g, axis=-1, keepdims=True), MIN_NORM)
    xp = x.reshape(x.shape[:-1] + (-1, 2))
    out = np.stack(
        [g[..., 0] * xp[..., 0] - g[..., 1] * xp[..., 1],
         g[..., 1] * xp[..., 0] + g[..., 0] * xp[..., 1]], axis=-1)
    return out.reshape(x.shape)


def prepare(u_idx, r_idx, v_idx, emb_entity, rel_diag, relation_bias_1,
            relation_bias_2, bias_head, bias_tail, sigma):
    """Host precompute: lhsT, per-(core,tile) rhs slices, affine terms."""
    emb = np.asarray(emb_entity, np.float64)
    u = np.asarray(u_idx).astype(np.int64)
    r = np.asarray(r_idx).astype(np.int64)
    v = np.asarray(v_idx).astype(np.int64)

    # head transform chain (exact, f64)
    head = _expmap0(emb[u])
    head = _mobius(head, _expmap0(np.asarray(relation_bias_1, np.float64)[r]))
    head = _givens(np.asarray(rel_diag, np.float64)[r], head)
    head = _mobius(head, _expmap0(np.asarray(relation_bias_2, np.float64)[r]))
    s_h = np.sum(head * head, axis=1)
    sig = 1.0 / (1.0 + np.exp(-np.asarray(sigma, np.float64)[r]))
    C = (MARGIN + np.asarray(bias_head, np.float64)[u]
         + (1 - sig) * np.log(np.maximum(1 - s_h, MIN_NORM)))

    w = (-SQS * head).astype(E4M3)
    srow = np.sum((w.astype(np.float64) / SQS) ** 2, axis=1) * SCALE
    lhs = np.zeros((P, B), E4M3)
    lhs[0:64, :] = w.T
    lhs[64:128, :] = w.T

    # entity-side constants
    rn = np.linalg.norm(emb, axis=1)
    un = np.maximum(rn, MIN_NORM)
    th = np.tanh(un)
    A = 2.0 * th / un
    xq = (SQS * A[:, None] * emb).astype(E4M3)
    xq64 = xq.astype(np.float64)
    t2 = th * th
    t2s = np.maximum(t2 * SCALE,
                     np.sum((xq64 / SQS) ** 2, axis=1) / 4 * SCALE
                     ).astype(np.float64)
    LE = np.log(np.maximum(1 - t2, MIN_NORM))

    # per-(core, batch-tile) unique entity columns
    c_of = v // SHARD
    U = [[None] * NBT for _ in range(NCORES)]
    maxu = 0
    for t in range(NBT):
        sub = v[t * P:(t + 1) * P].ravel()
        cs = c_of[t * P:(t + 1) * P].ravel()
        for c in range(NCORES):
            uu = np.unique(sub[cs == c])
            U[c][t] = uu
            maxu = max(maxu, len(uu))
    l_bt = max(GW, math.ceil(maxu / 1024) * 1024)

    xqT = np.ascontiguousarray(xq.T)            # [64, N_ENT] fp8
    in_maps = []
    for c in range(NCORES):
        rhs = np.empty((NBT * P, l_bt // 2), E4M3)
        for t in range(NBT):
            uu = U[c][t]
            cols = np.full(l_bt, c * SHARD, np.int64)
            cols[:len(uu)] = uu
            xs = xqT[:, cols].reshape(64, l_bt // 1024, 2, 512)
            blk = rhs[t * P:(t + 1) * P]
            blk[0:64] = xs[:, :, 0, :].reshape(64, l_bt // 2)
            blk[64:128] = xs[:, :, 1, :].reshape(64, l_bt // 2)
        in_maps.append({"lhs": lhs, "rhs": rhs})

    aux = dict(U=U, v=v, C=C, sig=sig, LE=LE, t2s=t2s, srow=srow,
               bias_tail=np.asarray(bias_tail, np.float64), l_bt=l_bt)
    return in_maps, aux


def assemble(results, aux):
    v, U = aux["v"], aux["U"]
    sel = np.empty((B, NCAND), np.float32)
    c_of = v // SHARD
    for c in range(NCORES):
        O = np.asarray(results[c]["out"]).astype(np.float32)  # [NBT*P, l_bt]
        for t in range(NBT):
            rows = slice(t * P, (t + 1) * P)
            mask = c_of[rows] == c
            if not mask.any():
                continue
            rr, cc = np.nonzero(mask)
            j = np.searchsorted(U[c][t], v[rows][mask])
            sel[t * P + rr, cc] = O[t * P + rr, j]
    n2 = np.maximum(
        (sel + aux["srow"][:, None] + aux["t2s"][v]) / SCALE, MIN_NORM)
    score = (aux["C"][:, None] + aux["sig"][:, None] * aux["LE"][v]
             + aux["bias_tail"][v] - np.log(n2))
    return score.astype(np.float32)


def kernel(**inputs) -> np.ndarray:
    in_maps, aux = prepare(**inputs)
    nc = get_module(aux["l_bt"])
    res = bass_utils.run_bass_kernel_spmd(
        nc, in_maps, core_ids=list(range(NCORES))
    )
    return assemble(res.results, aux)
